# revision 1
# baseline (speedup 1.0000x reference)
"""DeepSeekV3 MoE gate (256 experts, top-8, 8 groups/top-4, sigmoid scoring,
seq-aux-loss) as a Bass/Tile kernel on 8 Trainium2 NeuronCores.

Strategy (data parallel over tokens, per the sharding hint):
  - 8192 tokens are split 1024/core (cores 0-3 hold batch row 0, 4-7 row 1);
    the [256, 7168] gate weight + [256] bias are replicated.
  - Per core, logits = x @ W.T are computed as three bf16 matmul products
    (xh@wh + xl@wh + xh@wl with x = xh + xl, W = wh + wl hi/lo bf16 splits)
    accumulated in fp32 PSUM — full-rate PE with ~fp32 accuracy.  The host
    pre-splits and pre-transposes x/W into PE-ready [h, t]/[h, e] tile
    layouts so no on-device transposes are needed.
  - Routing per 128-token tile entirely on-chip: sigmoid (ScalarE LUT),
    +bias, grouped top-2-sum via the DVE Max8 unit, top-4 group threshold +
    mask, masked top-8 values/indices (Max8 + MaxIndex — identical ordering
    and tie-break semantics to jax.lax.top_k), normalize, scale.
  - Aux-loss needs only two per-expert sums over each batch row (selection
    counts and sigmoid-score sums); each core reduces its tokens via a
    1-row fp16 matmul into PSUM and the host combines the 8 [2,256]
    partials into the scalar (the "all-reduce only the aux scalar" step).
"""

import numpy as np
import ml_dtypes

BF16 = ml_dtypes.bfloat16

E = 256        # experts
H = 7168       # hidden dim
G = 8          # groups
EG = E // G    # experts per group
TOPK = 8
TOPKG = 4
HC = H // 128  # contraction chunks
ROUTE_SCALE = 2.5
AUX_ALPHA = 0.001
NCORES = 8
NT = 8         # 128-token tiles per core

_cache = {}


def _build_module():
    from concourse import bacc, tile, mybir

    f32 = mybir.dt.float32
    bf16 = mybir.dt.bfloat16
    f16 = mybir.dt.float16
    u32 = mybir.dt.uint32
    AF = mybir.ActivationFunctionType
    OP = mybir.AluOpType
    AX = mybir.AxisListType

    nc = bacc.Bacc("TRN2", target_bir_lowering=False, debug=False)

    xh_d = nc.dram_tensor("xh", [NT, 128, HC, 128], bf16, kind="ExternalInput")
    xl_d = nc.dram_tensor("xl", [NT, 128, HC, 128], bf16, kind="ExternalInput")
    wc_d = nc.dram_tensor("wc", [128, HC, 512], bf16, kind="ExternalInput")
    br_d = nc.dram_tensor("biasr", [128, E], f32, kind="ExternalInput")
    wout_d = nc.dram_tensor("wout", [NT, 128, TOPK], f32, kind="ExternalOutput")
    iout_d = nc.dram_tensor("iout", [NT, 128, TOPK], u32, kind="ExternalOutput")
    aux_d = nc.dram_tensor("aux", [1, 2 * E], f32, kind="ExternalOutput")

    with tile.TileContext(nc) as tc:
        with (
            tc.tile_pool(name="wpool", bufs=1) as wpool,
            tc.tile_pool(name="xpool", bufs=2) as xpool,
            tc.tile_pool(name="spool", bufs=2) as spool,
            tc.tile_pool(name="opool", bufs=2) as opool,
            tc.tile_pool(name="cpool", bufs=1) as cpool,
            tc.tile_pool(name="psum", bufs=2, space="PSUM") as psum_pool,
            tc.tile_pool(name="psum_aux", bufs=1, space="PSUM") as paux_pool,
        ):
            # w DMA split into chunk-slices interleaved with tile-0 x slices
            # in PE consumption order, so the first matmuls start early.
            WSPLIT = 8
            XSPLIT = 4
            wstep = HC // WSPLIT
            xstep = HC // XSPLIT
            w_sb = wpool.tile([128, HC, 512], bf16)
            x0h = xpool.tile([128, HC, 128], bf16, tag="xh")
            x0l = xpool.tile([128, HC, 128], bf16, tag="xl")
            for s in range(WSPLIT):
                lo, hi = s * wstep, min(HC, (s + 1) * wstep)
                nc.sync.dma_start(w_sb[:, lo:hi, :], wc_d[:, lo:hi, :])
                if s % 2 == 0:
                    xs = s // 2
                    xlo = xs * xstep
                    xhi = HC if xs == XSPLIT - 1 else (xs + 1) * xstep
                    nc.sync.dma_start(x0h[:, xlo:xhi, :], xh_d[0, :, xlo:xhi, :])
                    nc.sync.dma_start(x0l[:, xlo:xhi, :], xl_d[0, :, xlo:xhi, :])
            bias_r = cpool.tile([128, E], f32)
            nc.sync.dma_start(bias_r[:], br_d[:])
            ones_h = cpool.tile([128, 1], f16)
            nc.vector.memset(ones_h[:], 1.0)

            sall_ps = paux_pool.tile([1, 2 * E], f32, tag="sall")

            for n in range(NT):
                if n == 0:
                    xh_t, xl_t = x0h, x0l
                else:
                    xh_t = xpool.tile([128, HC, 128], bf16, tag="xh")
                    xl_t = xpool.tile([128, HC, 128], bf16, tag="xl")
                    for s in range(XSPLIT):
                        lo = s * xstep
                        hi = HC if s == XSPLIT - 1 else (s + 1) * xstep
                        nc.sync.dma_start(xh_t[:, lo:hi, :], xh_d[n, :, lo:hi, :])
                        nc.sync.dma_start(xl_t[:, lo:hi, :], xl_d[n, :, lo:hi, :])

                # logits: P1[:, 0:256] = xh@wh + xl@wh, P2[:, 256:512] = xh@wl
                ps = psum_pool.tile([128, 512], f32, tag="logits")
                for c in range(HC):
                    nc.tensor.matmul(
                        ps[:, :], lhsT=xh_t[:, c, :], rhs=w_sb[:, c, :],
                        start=(c == 0), stop=False,
                    )
                    nc.tensor.matmul(
                        ps[:, 0:E], lhsT=xl_t[:, c, :], rhs=w_sb[:, c, 0:E],
                        start=False, stop=(c == HC - 1),
                    )

                p2c = spool.tile([128, E], f32, tag="p2c")
                nc.scalar.copy(p2c[:], ps[:, E : 2 * E])
                logits = spool.tile([128, E], f32, tag="logits_sb")
                nc.vector.tensor_add(logits[:], ps[:, 0:E], p2c[:])

                scores = spool.tile([128, E], f32, tag="scores")
                nc.scalar.activation(scores[:], logits[:], AF.Sigmoid)
                scores_b = spool.tile([128, E], f32, tag="scores_b")
                nc.vector.tensor_add(scores_b[:], scores[:], bias_r[:])

                # per-group top-2 sum -> group scores
                top8g = spool.tile([128, G, 8], f32, tag="top8g")
                for g in range(G):
                    nc.vector.max(top8g[:, g, :], scores_b[:, g * EG : (g + 1) * EG])
                gsc = spool.tile([128, G], f32, tag="gsc")
                nc.vector.reduce_sum(gsc[:], top8g[:, :, 0:2], axis=AX.X)

                # top-4 groups -> expert mask
                g8 = spool.tile([128, 8], f32, tag="g8")
                nc.vector.max(g8[:], gsc[:])
                gmask = spool.tile([128, G, 1], f32, tag="gmask")
                nc.vector.tensor_scalar(
                    gmask[:, :, 0], gsc[:], g8[:, 3:4], None, op0=OP.is_ge
                )
                masked = spool.tile([128, G, EG], f32, tag="masked")
                nc.vector.tensor_tensor(
                    masked[:],
                    scores_b[:].rearrange("p (g e) -> p g e", g=G),
                    gmask[:].broadcast_to([128, G, EG]),
                    op=OP.mult,
                )
                masked2 = masked[:].rearrange("p g e -> p (g e)")

                # top-8 values (desc) + indices (lowest-index tie-break)
                v8 = spool.tile([128, 8], f32, tag="v8")
                nc.vector.max(v8[:], masked2)
                i8 = spool.tile([128, 8], u32, tag="i8")
                nc.vector.max_index(i8[:], v8[:], masked2)

                # weights = v8 / sum(v8) * ROUTE_SCALE
                s8 = spool.tile([128, 1], f32, tag="s8")
                nc.vector.reduce_sum(s8[:], v8[:], axis=AX.X)
                r8 = spool.tile([128, 1], f32, tag="r8")
                nc.vector.reciprocal(r8[:], s8[:])
                wt = opool.tile([128, 8], f32, tag="wt")
                nc.vector.tensor_scalar(
                    wt[:], v8[:], r8[:, 0:1], ROUTE_SCALE, op0=OP.mult, op1=OP.mult
                )
                nc.sync.dma_start(wout_d[n], wt[:])
                nc.sync.dma_start(iout_d[n], i8[:])

                # aux partials: one fp16 matmul accumulates [score_sums | sel_counts]
                aux_in = spool.tile([128, 2, E], f16, tag="aux_in")
                nc.vector.tensor_copy(aux_in[:, 0, :], scores[:])
                nc.vector.tensor_scalar(
                    aux_in[:, 1, :], masked2, v8[:, 7:8], None, op0=OP.is_ge
                )
                nc.tensor.matmul(
                    sall_ps[:], lhsT=ones_h[:],
                    rhs=aux_in[:].rearrange("p a e -> p (a e)"),
                    start=(n == 0), stop=(n == NT - 1),
                )

            aux_sb = opool.tile([1, 2 * E], f32, tag="aux_sb")
            nc.vector.tensor_copy(aux_sb[:], sall_ps[:])
            nc.sync.dma_start(aux_d[:], aux_sb[:])

    nc.compile()
    return nc


def _get_module():
    if "nc" not in _cache:
        _cache["nc"] = _build_module()
    return _cache["nc"]


def _host_prep_x(x2):
    """x2 [T, H] f32 -> per-core (xh, xl) [NT, 128, HC, 128] bf16 tile layouts."""
    tpc = x2.shape[0] // NCORES
    xh = x2.astype(BF16)
    xl = (x2 - xh.astype(np.float32)).astype(BF16)
    out = []
    for c in range(NCORES):
        sl = slice(c * tpc, (c + 1) * tpc)
        pair = []
        for a in (xh[sl], xl[sl]):
            b = a.reshape(NT, 128, HC, 128).transpose(0, 3, 2, 1)
            pair.append(np.ascontiguousarray(b))
        out.append(pair)
    return out


def _host_prep_w(w):
    """w [E, H] f32 -> [128, HC, 512] bf16 ([..., :256]=wh.T, [..., 256:]=wl.T)."""
    wh = w.astype(BF16)
    wl = (w - wh.astype(np.float32)).astype(BF16)
    wc = np.empty((128, HC, 2 * E), dtype=BF16)
    wc[:, :, :E] = wh.reshape(E, HC, 128).transpose(2, 1, 0)
    wc[:, :, E:] = wl.reshape(E, HC, 128).transpose(2, 1, 0)
    return np.ascontiguousarray(wc)


def kernel(x, gate_weight, bias):
    from concourse.bass_utils import run_bass_kernel_spmd

    nc = _get_module()
    x = np.asarray(x)
    gate_weight = np.asarray(gate_weight)
    bias = np.asarray(bias)
    B, S, _ = x.shape
    x2 = x.reshape(B * S, H)

    per_core = _host_prep_x(x2)
    wc = _host_prep_w(gate_weight)
    br = np.ascontiguousarray(
        np.broadcast_to(bias.astype(np.float32)[None, :], (128, E))
    )
    in_maps = [{"xh": xh, "xl": xl, "wc": wc, "biasr": br} for xh, xl in per_core]

    res = run_bass_kernel_spmd(nc, in_maps, core_ids=list(range(NCORES)))

    weights = np.concatenate(
        [r["wout"].reshape(-1, TOPK) for r in res.results], axis=0
    ).astype(np.float32)
    indices = np.ascontiguousarray(
        np.concatenate([r["iout"].reshape(-1, TOPK) for r in res.results], axis=0)
        .view(np.int32)
    ).astype(np.int32)

    # aux loss from per-core [score_sums | sel_counts] partials
    cpb = NCORES // B
    terms = []
    for b in range(B):
        acc = np.zeros(2 * E, np.float64)
        for c in range(cpb * b, cpb * (b + 1)):
            acc += res.results[c]["aux"].reshape(-1)
        ssc, ssel = acc[:E], acc[E:]
        terms.append(float((ssel * ssc).sum() / ((S * TOPK / E) * S)))
    aux_loss = np.float32(np.mean(terms) * AUX_ALPHA)

    return weights, indices, aux_loss
